# revision 6
# baseline (speedup 1.0000x reference)
"""Trainium2 Bass kernel for y = inputs @ weights.T + bias.

Shapes: inputs [8192, 4096] f32, weights [4096, 4096] f32, bias [4096] f32,
output [8192, 4096] f32.

Strategy (v2):
- Data-parallel across 8 NeuronCores: each core computes 1024 rows of the
  output; weights/bias are replicated.
- bf16 compute: host rounds x and w to bf16 (matmul rel err ~3e-3, far under
  the 2e-2 gate). Same PE rate as f32r (1 cycle/row) but half the DMA
  traffic (w alone is 67MB/core in f32) and FWL fast weight loads.
- Transposed product: stationary = w tile [128k, 128o], moving = x
  [128k, 512m], PSUM tile = [128 o-partitions, 512 m]. Bias is then a
  per-partition scalar [128,1], so PSUM eviction runs on BOTH the Scalar
  (ACT activation bias add) and Vector engines, halving drain chains.
- Output stored as yT [4096, 1024] bf16 per core; host transposes/upcasts.
- Phase 1 (x still streaming in): process ob 0-3 together, ko-outer, so
  each x slab is consumed the moment it lands (all 8 PSUM banks).
- Phase 2 (x resident): ob-sequential, PSUM bank pairs cycle with 4-ob
  pipelining depth.
- PE warm-up: dummy matmuls on a zeroed SBUF tile fill the DMA lead-in so
  the HAM clock gate is at 2.4 GHz when real matmuls start.
"""

import numpy as np
import ml_dtypes

import concourse.bacc as bacc
import concourse.mybir as mybir
import concourse.tile as tile
from concourse.bass_utils import run_bass_kernel_spmd

N_CORES = 8
N_FULL = 8192  # input rows
K_DIM = 4096  # contraction (in features)
O_DIM = 4096  # out features
M = N_FULL // N_CORES  # rows per core (1024)
P = 128
KO = K_DIM // P  # 32 k-slabs
OB = O_DIM // P  # 32 output-row blocks
N_TILE = 512  # moving free dim per matmul (1 PSUM bank of fp32)
MC = M // N_TILE  # 2 moving chunks per core
PH1_OBS = 4  # obs processed ko-outer while x streams in
N_DUMMY = 56  # warm-up matmuls (256-wide) to cover the DMA lead-in

_nc_cache = None


def _build():
    nc = bacc.Bacc(target_bir_lowering=False)

    xT = nc.dram_tensor("xT", [K_DIM, M], mybir.dt.bfloat16, kind="ExternalInput")
    wT = nc.dram_tensor("wT", [K_DIM, O_DIM], mybir.dt.bfloat16, kind="ExternalInput")
    biasT = nc.dram_tensor("biasT", [P, OB], mybir.dt.float32, kind="ExternalInput")
    yT = nc.dram_tensor("yT", [O_DIM, M], mybir.dt.bfloat16, kind="ExternalOutput")

    xT3 = xT.ap().rearrange("(ko p) m -> p ko m", p=P)
    wT3 = wT.ap().rearrange("(ko p) o -> p ko o", p=P)
    yT3 = yT.ap().rearrange("(ob p) m -> p ob m", p=P)

    with tile.TileContext(nc) as tc:
        with (
            tc.tile_pool(name="persist", bufs=1) as persist,
            tc.tile_pool(name="wpool", bufs=4) as wpool,
            tc.tile_pool(name="opool", bufs=10) as opool,
            tc.tile_pool(name="psum", bufs=1, space="PSUM") as psum_pool,
        ):
            # --- PE warm-up: dummies on a zeroed tile, result never read.
            dummy_sb = persist.tile([P, 384], mybir.dt.bfloat16, tag="dummy")
            nc.gpsimd.memset(dummy_sb[:], 0)
            # Dummy PSUM shares bank tag ps7: its writes finish long before
            # the first real user of ps7 (phase-1 ob3/mc1) issues.
            dummy_ps = psum_pool.tile([P, N_TILE], mybir.dt.float32, tag="ps7")
            for _ in range(N_DUMMY):
                nc.tensor.matmul(
                    dummy_ps[:, :256],
                    dummy_sb[:, :128],
                    dummy_sb[:, 128:384],
                    start=True,
                    stop=True,
                )

            # --- bias [128, 32] f32
            bias_sb = persist.tile([P, OB], mybir.dt.float32, tag="bias")
            nc.gpsimd.dma_start(bias_sb[:], biasT.ap()[:])

            # --- x preload: 32 slabs [128, 1024] bf16 (256KB each).
            # ko0 lands in two 512-col chunks so the first matmul only waits
            # for 128KB.
            x_sb = []
            x_engs = [nc.gpsimd, nc.scalar]
            xq = 0
            for ko in range(KO):
                x_t = persist.tile([P, M], mybir.dt.bfloat16, tag=f"x{ko}")
                nchunk = 2 if ko < 4 else 1
                csz = M // nchunk
                for c in range(nchunk):
                    x_engs[xq % 2].dma_start(
                        x_t[:, c * csz : (c + 1) * csz],
                        xT3[:, ko, c * csz : (c + 1) * csz],
                    )
                    xq += 1
                x_sb.append(x_t)

            # --- w stream: one tile per ob, [128, 32ko, 128o] bf16 (1MB).
            # ob0 split by ko so MM(ko=0) waits only for 32KB.
            def load_w(ob):
                w_t = wpool.tile([P, KO, P], mybir.dt.bfloat16, tag="w", name="w_t")
                nc.sync.dma_start(w_t[:], wT3[:, :, ob * P : (ob + 1) * P])
                return w_t

            w_tiles = {}
            for ob in range(PH1_OBS):
                w_tiles[ob] = wpool.tile(
                    [P, KO, P], mybir.dt.bfloat16, tag="w", name="w_t"
                )
            # ko-interleaved delivery matched to phase-1 consumption order
            KOC = 2
            for koc in range(0, KO, KOC):
                for ob in range(PH1_OBS):
                    nc.sync.dma_start(
                        w_tiles[ob][:, koc : koc + KOC, :],
                        wT3[:, koc : koc + KOC, ob * P : (ob + 1) * P],
                    )

            def evict(ps_t, ob, mc, eng_i):
                o_t = opool.tile([P, N_TILE], mybir.dt.bfloat16, tag="o", name="o_t")
                if eng_i % 2 == 0:
                    nc.scalar.add(o_t[:], ps_t[:], bias_sb[:, ob : ob + 1])
                else:
                    nc.vector.tensor_scalar_add(o_t[:], ps_t[:], bias_sb[:, ob : ob + 1])
                if ob >= OB - 2:
                    oeng = nc.sync if eng_i % 2 == 0 else nc.scalar
                else:
                    oeng = nc.sync if eng_i % 2 == 0 else nc.gpsimd
                oeng.dma_start(yT3[:, ob, mc * N_TILE : (mc + 1) * N_TILE], o_t[:])

            # --- Phase 1: obs 0..3 ko-outer (8 PSUM banks), consuming each x
            # slab as it lands.
            ps1 = {
                (ob, mc): psum_pool.tile(
                    [P, N_TILE],
                    mybir.dt.float32,
                    tag=f"ps{2 * ob + mc}",
                    name=f"ps{2 * ob + mc}",
                )
                for ob in range(PH1_OBS)
                for mc in range(MC)
            }
            for ko in range(KO):
                for ob in range(PH1_OBS):
                    for mc in range(MC):
                        nc.tensor.matmul(
                            ps1[(ob, mc)][:],
                            w_tiles[ob][:, ko, :],
                            x_sb[ko][:, mc * N_TILE : (mc + 1) * N_TILE],
                            start=(ko == 0),
                            stop=(ko == KO - 1),
                        )
            # prefetch w for the next obs before the eviction burst
            for ob in range(PH1_OBS, 2 * PH1_OBS):
                w_tiles[ob] = load_w(ob)
            for ob in range(PH1_OBS):
                for mc in range(MC):
                    evict(ps1[(ob, mc)], ob, mc, 2 * ob + mc)

            # --- Phase 2: remaining obs sequential, bank pairs cycle mod 4.
            for ob in range(PH1_OBS, OB):
                if ob not in w_tiles:
                    w_tiles[ob] = load_w(ob)
                if ob + 1 < OB and (ob + 1) not in w_tiles:
                    w_tiles[ob + 1] = load_w(ob + 1)
                ps = [
                    psum_pool.tile(
                        [P, N_TILE],
                        mybir.dt.float32,
                        tag=f"ps{2 * (ob % PH1_OBS) + mc}",
                        name=f"ps{2 * (ob % PH1_OBS) + mc}",
                    )
                    for mc in range(MC)
                ]
                for ko in range(KO):
                    for mc in range(MC):
                        nc.tensor.matmul(
                            ps[mc][:],
                            w_tiles[ob][:, ko, :],
                            x_sb[ko][:, mc * N_TILE : (mc + 1) * N_TILE],
                            start=(ko == 0),
                            stop=(ko == KO - 1),
                        )
                del w_tiles[ob]
                for mc in range(MC):
                    evict(ps[mc], ob, mc, mc)

    nc.compile()
    return nc


def _get_nc():
    global _nc_cache
    if _nc_cache is None:
        _nc_cache = _build()
    return _nc_cache


def _make_in_maps(inputs, weights, bias):
    x = np.asarray(inputs, dtype=np.float32)
    w = np.asarray(weights, dtype=np.float32)
    b = np.asarray(bias, dtype=np.float32)

    xT = np.ascontiguousarray(x.T).astype(ml_dtypes.bfloat16)  # [K, N_FULL]
    wT = np.ascontiguousarray(w.T).astype(ml_dtypes.bfloat16)  # [K, O]
    bT = np.ascontiguousarray(b.reshape(OB, P).T)  # [128, 32]

    in_maps = []
    for c in range(N_CORES):
        xTc = np.ascontiguousarray(xT[:, c * M : (c + 1) * M])
        in_maps.append({"xT": xTc, "wT": wT, "biasT": bT})
    return in_maps


def _assemble(res):
    outs = []
    for r in res.results:
        yTc = np.asarray(r["yT"])  # [O, M] bf16
        outs.append(yTc.astype(np.float32).T)  # [M, O] f32
    return np.ascontiguousarray(np.concatenate(outs, axis=0))


def kernel(**inputs):
    nc = _get_nc()
    in_maps = _make_in_maps(inputs["inputs"], inputs["weights"], inputs["bias"])
    res = run_bass_kernel_spmd(nc, in_maps, core_ids=list(range(N_CORES)))
    return _assemble(res)


def run_traced(inputs, weights, bias, **trace_kwargs):
    """Used by test.py: same computation, returns (output, BassKernelResults)."""
    nc = _get_nc()
    in_maps = _make_in_maps(inputs, weights, bias)
    res = run_bass_kernel_spmd(
        nc, in_maps, core_ids=list(range(N_CORES)), trace=True, **trace_kwargs
    )
    return _assemble(res), res


# revision 8
# speedup vs baseline: 1.0333x; 1.0333x over previous
"""Trainium2 Bass kernel for y = inputs @ weights.T + bias.

Shapes: inputs [8192, 4096] f32, weights [4096, 4096] f32, bias [4096] f32,
output [8192, 4096] f32.

Strategy (v2):
- Data-parallel across 8 NeuronCores: each core computes 1024 rows of the
  output; weights/bias are replicated.
- bf16 compute: host rounds x and w to bf16 (matmul rel err ~3e-3, far under
  the 2e-2 gate). Same PE rate as f32r (1 cycle/row) but half the DMA
  traffic (w alone is 67MB/core in f32) and FWL fast weight loads.
- Transposed product: stationary = w tile [128k, 128o], moving = x
  [128k, 512m], PSUM tile = [128 o-partitions, 512 m]. Bias is then a
  per-partition scalar [128,1], so PSUM eviction runs on BOTH the Scalar
  (ACT activation bias add) and Vector engines, halving drain chains.
- Output stored as yT [4096, 1024] bf16 per core; host transposes/upcasts.
- Phase 1 (x still streaming in): process ob 0-3 together, ko-outer, so
  each x slab is consumed the moment it lands (all 8 PSUM banks).
- Phase 2 (x resident): ob-sequential, PSUM bank pairs cycle with 4-ob
  pipelining depth.
- PE warm-up: dummy matmuls on a zeroed SBUF tile fill the DMA lead-in so
  the HAM clock gate is at 2.4 GHz when real matmuls start.
"""

import numpy as np
import ml_dtypes

import concourse.bacc as bacc
import concourse.mybir as mybir
import concourse.tile as tile
from concourse.bass_utils import run_bass_kernel_spmd

N_CORES = 8
N_FULL = 8192  # input rows
K_DIM = 4096  # contraction (in features)
O_DIM = 4096  # out features
M = N_FULL // N_CORES  # rows per core (1024)
P = 128
KO = K_DIM // P  # 32 k-slabs
OB = O_DIM // P  # 32 output-row blocks
N_TILE = 512  # moving free dim per matmul (1 PSUM bank of fp32)
MC = M // N_TILE  # 2 moving chunks per core
PH1_OBS = 4  # obs processed ko-outer while x streams in
N_DUMMY = 56  # warm-up matmuls (256-wide) to cover the DMA lead-in

_nc_cache = None


def _build():
    nc = bacc.Bacc(target_bir_lowering=False)

    xT = nc.dram_tensor("xT", [K_DIM, M], mybir.dt.bfloat16, kind="ExternalInput")
    wH = nc.dram_tensor("wH", [P, OB, KO, P], mybir.dt.bfloat16, kind="ExternalInput")
    biasT = nc.dram_tensor("biasT", [P, OB], mybir.dt.float32, kind="ExternalInput")
    yT = nc.dram_tensor("yT", [O_DIM, M], mybir.dt.bfloat16, kind="ExternalOutput")

    xT3 = xT.ap().rearrange("(ko p) m -> p ko m", p=P)
    w4 = wH.ap()
    yT3 = yT.ap().rearrange("(ob p) m -> p ob m", p=P)

    with tile.TileContext(nc) as tc:
        with (
            tc.tile_pool(name="persist", bufs=1) as persist,
            tc.tile_pool(name="wpool", bufs=6) as wpool,
            tc.tile_pool(name="opool", bufs=10) as opool,
            tc.tile_pool(name="psum", bufs=1, space="PSUM") as psum_pool,
        ):
            # --- PE warm-up: dummies on a zeroed tile, result never read.
            dummy_sb = persist.tile([P, 384], mybir.dt.bfloat16, tag="dummy")
            nc.gpsimd.memset(dummy_sb[:], 0)
            # Dummy PSUM shares bank tag ps7: its writes finish long before
            # the first real user of ps7 (phase-1 ob3/mc1) issues.
            dummy_ps = psum_pool.tile([P, N_TILE], mybir.dt.float32, tag="ps7")
            for _ in range(N_DUMMY):
                nc.tensor.matmul(
                    dummy_ps[:, :256],
                    dummy_sb[:, :128],
                    dummy_sb[:, 128:384],
                    start=True,
                    stop=True,
                )

            # --- bias [128, 32] f32
            bias_sb = persist.tile([P, OB], mybir.dt.float32, tag="bias")
            nc.gpsimd.dma_start(bias_sb[:], biasT.ap()[:])

            # --- x preload: 32 slabs [128, 1024] bf16 (256KB each).
            # ko0 lands in two 512-col chunks so the first matmul only waits
            # for 128KB.
            x_sb = []
            x_engs = [nc.gpsimd, nc.scalar]
            xq = 0
            for ko in range(KO):
                x_t = persist.tile([P, M], mybir.dt.bfloat16, tag=f"x{ko}")
                nchunk = 2 if ko < 4 else 1
                csz = M // nchunk
                for c in range(nchunk):
                    x_engs[xq % 2].dma_start(
                        x_t[:, c * csz : (c + 1) * csz],
                        xT3[:, ko, c * csz : (c + 1) * csz],
                    )
                    xq += 1
                x_sb.append(x_t)

            # --- w stream: one tile per ob, [128, 32ko, 128o] bf16 (1MB).
            # ob0 split by ko so MM(ko=0) waits only for 32KB.
            def load_w(ob):
                w_t = wpool.tile([P, KO, P], mybir.dt.bfloat16, tag="w", name="w_t")
                nc.sync.dma_start(w_t[:], w4[:, ob, :, :])
                return w_t

            w_tiles = {}
            for ob in range(PH1_OBS):
                w_tiles[ob] = wpool.tile(
                    [P, KO, P], mybir.dt.bfloat16, tag="w", name="w_t"
                )
            # ko-interleaved delivery matched to phase-1 consumption order
            KOC = 4
            for koc in range(0, KO, KOC):
                for ob in range(PH1_OBS):
                    nc.sync.dma_start(
                        w_tiles[ob][:, koc : koc + KOC, :],
                        w4[:, ob, koc : koc + KOC, :],
                    )

            def evict(ps_t, ob, mc, eng_i):
                o_t = opool.tile([P, N_TILE], mybir.dt.bfloat16, tag="o", name="o_t")
                if eng_i % 2 == 0:
                    nc.scalar.add(o_t[:], ps_t[:], bias_sb[:, ob : ob + 1])
                else:
                    nc.vector.tensor_scalar_add(o_t[:], ps_t[:], bias_sb[:, ob : ob + 1])
                if ob >= OB - 2:
                    oeng = nc.sync if eng_i % 2 == 0 else nc.scalar
                else:
                    oeng = nc.sync if eng_i % 2 == 0 else nc.gpsimd
                oeng.dma_start(yT3[:, ob, mc * N_TILE : (mc + 1) * N_TILE], o_t[:])

            # --- Phase 1: obs 0..3 ko-outer (8 PSUM banks), consuming each x
            # slab as it lands.
            ps1 = {
                (ob, mc): psum_pool.tile(
                    [P, N_TILE],
                    mybir.dt.float32,
                    tag=f"ps{2 * ob + mc}",
                    name=f"ps{2 * ob + mc}",
                )
                for ob in range(PH1_OBS)
                for mc in range(MC)
            }
            for ko in range(KO):
                for ob in range(PH1_OBS):
                    for mc in range(MC):
                        nc.tensor.matmul(
                            ps1[(ob, mc)][:],
                            w_tiles[ob][:, ko, :],
                            x_sb[ko][:, mc * N_TILE : (mc + 1) * N_TILE],
                            start=(ko == 0),
                            stop=(ko == KO - 1),
                        )
            # prefetch w for the next obs before the eviction burst
            for ob in range(PH1_OBS, 2 * PH1_OBS):
                w_tiles[ob] = load_w(ob)
            for ob in range(PH1_OBS):
                for mc in range(MC):
                    evict(ps1[(ob, mc)], ob, mc, 2 * ob + mc)

            # --- Phase 2: remaining obs sequential, bank pairs cycle mod 4.
            for ob in range(PH1_OBS, OB):
                if ob not in w_tiles:
                    w_tiles[ob] = load_w(ob)
                for pf in (ob + 1, ob + 2, ob + 3):
                    if pf < OB and pf not in w_tiles:
                        w_tiles[pf] = load_w(pf)
                ps = [
                    psum_pool.tile(
                        [P, N_TILE],
                        mybir.dt.float32,
                        tag=f"ps{2 * (ob % PH1_OBS) + mc}",
                        name=f"ps{2 * (ob % PH1_OBS) + mc}",
                    )
                    for mc in range(MC)
                ]
                for ko in range(KO):
                    for mc in range(MC):
                        nc.tensor.matmul(
                            ps[mc][:],
                            w_tiles[ob][:, ko, :],
                            x_sb[ko][:, mc * N_TILE : (mc + 1) * N_TILE],
                            start=(ko == 0),
                            stop=(ko == KO - 1),
                        )
                del w_tiles[ob]
                for mc in range(MC):
                    evict(ps[mc], ob, mc, mc)

    nc.compile()
    return nc


def _get_nc():
    global _nc_cache
    if _nc_cache is None:
        _nc_cache = _build()
    return _nc_cache


def _make_in_maps(inputs, weights, bias):
    x = np.asarray(inputs, dtype=np.float32)
    w = np.asarray(weights, dtype=np.float32)
    b = np.asarray(bias, dtype=np.float32)

    xT = np.ascontiguousarray(x.T).astype(ml_dtypes.bfloat16)  # [K, N_FULL]
    wbf = w.T.astype(ml_dtypes.bfloat16)  # [K, O]
    # pre-tiled: wH[p, ob, ko, oo] = wT[128*ko+p, 128*ob+oo] -> 8KB/partition runs
    wH = np.ascontiguousarray(wbf.reshape(KO, P, OB, P).transpose(1, 2, 0, 3))
    bT = np.ascontiguousarray(b.reshape(OB, P).T)  # [128, 32]

    in_maps = []
    for c in range(N_CORES):
        xTc = np.ascontiguousarray(xT[:, c * M : (c + 1) * M])
        in_maps.append({"xT": xTc, "wH": wH, "biasT": bT})
    return in_maps


def _assemble(res):
    outs = []
    for r in res.results:
        yTc = np.asarray(r["yT"])  # [O, M] bf16
        outs.append(yTc.astype(np.float32).T)  # [M, O] f32
    return np.ascontiguousarray(np.concatenate(outs, axis=0))


def kernel(**inputs):
    nc = _get_nc()
    in_maps = _make_in_maps(inputs["inputs"], inputs["weights"], inputs["bias"])
    res = run_bass_kernel_spmd(nc, in_maps, core_ids=list(range(N_CORES)))
    return _assemble(res)


def run_traced(inputs, weights, bias, **trace_kwargs):
    """Used by test.py: same computation, returns (output, BassKernelResults)."""
    nc = _get_nc()
    in_maps = _make_in_maps(inputs, weights, bias)
    res = run_bass_kernel_spmd(
        nc, in_maps, core_ids=list(range(N_CORES)), trace=True, **trace_kwargs
    )
    return _assemble(res), res
